# revision 1
# baseline (speedup 1.0000x reference)
"""CapsNet dynamic-routing kernel for 8 Trainium2 NeuronCores.

Sharding: tensor-parallel over N_OUT (8 output capsules per core). x_hat
(B, N_OUT, N_IN, D_OUT) is never materialized; every contraction over it is
re-expressed against W and x as PE matmuls:

  s_t[b,o,do]     = sum_{i,di} W[o,i,do,di] * c_t[b,o,i] * x[b,i,di]
  beta_inc[b,o,i] = sum_di ( sum_do v[b,o,do] W[o,i,do,di] ) * x[b,i,di]

Per core, per routing iteration:
  1. Wv matmuls: lhsT = s^T slices (squash scale f applied later), rhs = W in
     [do,(di,i)] layout; 8 concurrent PE tiles via row(32g)+col(64h) packing.
  2. Fused DVE STT drain: p = (Wv_psum * f[b,o]) * x  (f enters as the
     per-partition scalar -> v itself never materializes).
  3. tree-add over di (contiguous 1152-blocks, di-major) -> beta.
  4. ACT exp -> e; Z = sum_local_o e via select-matrix matmul; AllReduce of
     the (64,1152) Z partials; 1/Z via reciprocal_approx_fast.
  5. PE-transpose e chunks; fused drain cT = eT * rZT (bf16).
  6. y = cT(di-bcast) * xT; col-tiled s-matmuls -> s psum [(g,do), h, b].
  7. squash factor f from s via Square + block-diag-ones matmul (n2), kept as
     [b,o] scalars, applied in the next STT / the final output multiply.

Softmax over o spans cores -> one AllReduce per iteration (iters 1, 2 only;
iteration-0 softmax of zeros is uniform 1/64, folded into the squash scale).
Output v is (64, 8, 16) per core; host concatenates over o.
"""

import os
import sys
import types

import numpy as np
import ml_dtypes

B = 64
N_IN = 1152
D_IN = 8
N_OUT = 64
D_OUT = 16
O_LOC = 8
N_CORES = 8
KD = N_IN * D_IN  # 9216
NCH = 9           # i chunks of 128
EPS = 1e-8

bf16 = ml_dtypes.bfloat16

_CACHE = {}
last_exec_ns = None


def _install_ntff_hook():
    try:
        import antenv
    except ImportError:
        return
    if "antenv.axon_hooks" in sys.modules:
        return
    mod = types.ModuleType("antenv.axon_hooks")
    _state = {"hook": None}
    mod.set_axon_ntff_profile_hook = lambda h: _state.__setitem__("hook", h)
    mod.get_axon_ntff_profile_hook = lambda: _state["hook"]
    sys.modules["antenv.axon_hooks"] = mod
    antenv.axon_hooks = mod
    try:
        from trn_agent_boot.trn_boot import _ntff_profile_via_ctypes
        hook = _ntff_profile_via_ctypes("/opt/axon/libaxon_pjrt.so")
        if hook is not None:
            mod.set_axon_ntff_profile_hook(hook)
    except Exception:
        pass


def _build():
    import concourse.bacc as bacc
    import concourse.tile as tile
    import concourse.mybir as mybir

    dt = mybir.dt
    Alu = mybir.AluOpType
    Act = mybir.ActivationFunctionType

    nc = bacc.Bacc("TRN2", target_bir_lowering=False, debug=False,
                   num_devices=N_CORES)

    # ---- DRAM I/O ----
    d_xt = nc.dram_tensor("xt", [NCH, 128, D_IN, B], dt.bfloat16,
                          kind="ExternalInput")
    d_wf = nc.dram_tensor("wf", [128, D_IN, NCH, O_LOC * D_OUT], dt.bfloat16,
                          kind="ExternalInput")
    d_wdo = nc.dram_tensor("wdo", [128, 2, KD], dt.bfloat16,
                           kind="ExternalInput")
    d_xb = nc.dram_tensor("xb", [B, KD], dt.bfloat16, kind="ExternalInput")
    d_sel = nc.dram_tensor("sel", [128, B], dt.bfloat16, kind="ExternalInput")
    d_ones = nc.dram_tensor("onesbd", [128, 4], dt.float32,
                            kind="ExternalInput")
    d_idf = nc.dram_tensor("identf", [128, 128], dt.float32,
                           kind="ExternalInput")
    d_idb = nc.dram_tensor("identb", [128, 64], dt.bfloat16,
                           kind="ExternalInput")
    d_out = nc.dram_tensor("yout", [B, 2, 128], dt.float32,
                           kind="ExternalOutput")

    with tile.TileContext(nc) as tc:
        with (
            tc.tile_pool(name="const", bufs=1) as constp,
            tc.tile_pool(name="big", bufs=1) as bigp,
            tc.tile_pool(name="small", bufs=2) as smallp,
            tc.tile_pool(name="ps", bufs=1, space="PSUM") as psp,
            tc.tile_pool(name="psS", bufs=1, space="PSUM") as psS,
            tc.tile_pool(name="dram", bufs=1, space="DRAM") as dramp,
        ):
            # ---------- constants / inputs ----------
            xt = constp.tile([128, NCH, D_IN, B], dt.bfloat16)
            nc.sync.dma_start(xt[:], d_xt.rearrange("c p d b -> p c d b"))
            wf = constp.tile([128, D_IN, NCH, O_LOC * D_OUT], dt.bfloat16)
            nc.sync.dma_start(wf[:], d_wf[:])
            wdo = constp.tile([128, 2, KD], dt.bfloat16)
            nc.sync.dma_start(wdo[:], d_wdo[:])
            xb2 = constp.tile([128, KD], dt.bfloat16)
            nc.sync.dma_start(xb2[:B, :], d_xb[:])
            nc.sync.dma_start(xb2[B:, :], d_xb[:])
            sel = constp.tile([128, B], dt.bfloat16)
            nc.sync.dma_start(sel[:], d_sel[:])
            onesbd = constp.tile([128, 4], dt.float32)
            nc.sync.dma_start(onesbd[:], d_ones[:])
            idf = constp.tile([128, 128], dt.float32)
            nc.sync.dma_start(idf[:], d_idf[:])
            idb = constp.tile([128, 64], dt.bfloat16)
            nc.sync.dma_start(idb[:], d_idb[:])

            # persistent state
            beta = [bigp.tile([128, N_IN], dt.float32, tag=f"beta{g}",
                              name=f"beta{g}")
                    for g in range(4)]
            sT = bigp.tile([128, 2, B], dt.bfloat16, tag="sT")
            sTf = bigp.tile([128, 2, B], dt.float32, tag="sTf")
            fT2 = bigp.tile([128, 4], dt.float32, tag="fT2")
            epst = bigp.tile([4, 1], dt.float32, tag="epst")
            nc.gpsimd.memset(epst[:], EPS)

            def s_matmuls(ps, rhs_of):
                n_acc = NCH * D_IN
                for o in range(O_LOC):
                    g, h = o % 4, o // 4
                    k = 0
                    for ic in range(NCH):
                        for di in range(D_IN):
                            nc.tensor.matmul(
                                ps[32 * g:32 * g + 16, h, :],
                                wf[:, di, ic, 16 * o:16 * o + 16],
                                rhs_of(o, ic, di),
                                start=(k == 0), stop=(k == n_acc - 1),
                                tile_position=(0, 32 * g),
                            )
                            k += 1

            def squash_from_ps(ps, first):
                sq = smallp.tile([128, 2, B], dt.float32, tag="sq")
                scale = (1.0 / N_OUT) if first else 1.0
                for h in range(2):
                    nc.vector.tensor_copy(sT[:, h, :], ps[:, h, :])
                    nc.vector.tensor_copy(sTf[:, h, :], ps[:, h, :])
                    nc.scalar.activation(sq[:, h, :], ps[:, h, :], Act.Square,
                                         scale=scale)
                n2ps = psp.tile([4, 2, B], dt.float32, tag="tiny")
                for h in range(2):
                    nc.tensor.matmul(n2ps[:, h, :], onesbd[:], sq[:, h, :],
                                     start=True, stop=True)
                n2 = smallp.tile([4, 2, B], dt.float32, tag="n2s")
                nc.vector.tensor_copy(n2[:], n2ps[:])
                a = smallp.tile([4, 2, B], dt.float32, tag="fa")
                srt = smallp.tile([4, 2, B], dt.float32, tag="fs")
                nc.scalar.activation(a[:], n2[:], Act.Copy, bias=1.0)
                nc.scalar.activation(srt[:], n2[:], Act.Sqrt, bias=epst[:])
                nc.vector.tensor_mul(a[:], a[:], srt[:])
                nc.vector.reciprocal(srt[:], a[:])
                nc.vector.tensor_mul(a[:], n2[:], srt[:])
                if first:
                    nc.vector.tensor_scalar_mul(a[:], a[:], 1.0 / N_OUT)
                fps = psp.tile([128, 4], dt.float32, tag="tiny2")
                nc.tensor.transpose(
                    fps[:], a.rearrange("g h b -> g (h b)"), idf[:4, :4])
                nc.vector.tensor_copy(fT2[:], fps[:])

            # ---------- iteration 0 ----------
            ps0 = psS.tile([128, 2, B], dt.float32, tag="sps")
            s_matmuls(ps0, lambda o, ic, di: xt[:, ic, di, :])
            squash_from_ps(ps0, first=True)

            zin = dramp.tile([B, N_IN], dt.float32)
            zout = dramp.tile([B, N_IN], dt.float32)

            for it in (1, 2):
                # ----- beta increment -----
                NJ = KD // 512
                for g in range(4):
                    pbuf = bigp.tile([128, KD], dt.bfloat16, tag="p")
                    for j in range(NJ):
                        wps = psp.tile([128, 512], dt.float32,
                                       tag=f"wv{j % 2}")
                        for h in range(2):
                            nc.tensor.matmul(
                                wps[64 * h:64 * h + 64, :],
                                sT[32 * g:32 * g + 16, h, :],
                                wdo[32 * g:32 * g + 16, h,
                                    512 * j:512 * (j + 1)],
                                start=True, stop=True,
                                tile_position=(32 * g, 64 * h),
                            )
                        # ACT drain with the squash scale f as per-partition
                        # scale; then cheap bf16 TT with x on DVE.
                        nc.scalar.activation(
                            pbuf[:, 512 * j:512 * (j + 1)], wps[:],
                            Act.Copy, scale=fT2[:, g:g + 1])
                        nc.vector.tensor_mul(
                            pbuf[:, 512 * j:512 * (j + 1)],
                            pbuf[:, 512 * j:512 * (j + 1)],
                            xb2[:, 512 * j:512 * (j + 1)])
                    pv = pbuf.rearrange("p (a two n) -> p a two n",
                                        a=4, two=2)
                    q1 = bigp.tile([128, 4, N_IN], dt.bfloat16, tag="q1")
                    nc.vector.tensor_add(q1[:], pv[:, :, 0, :],
                                         pv[:, :, 1, :])
                    q1v = q1.rearrange("p (a two) n -> p a two n", two=2)
                    q2 = bigp.tile([128, 2, N_IN], dt.bfloat16, tag="q2")
                    nc.vector.tensor_add(q2[:], q1v[:, :, 0, :],
                                         q1v[:, :, 1, :])
                    if it == 1:
                        nc.vector.tensor_add(beta[g][:], q2[:, 0, :],
                                             q2[:, 1, :])
                    else:
                        binc = bigp.tile([128, N_IN], dt.float32, tag="binc")
                        nc.vector.tensor_add(binc[:], q2[:, 0, :],
                                             q2[:, 1, :])
                        nc.vector.tensor_add(beta[g][:], beta[g][:],
                                             binc[:])

                # ----- softmax Z + allreduce -----
                e = [bigp.tile([128, N_IN], dt.bfloat16, tag=f"e{g}", name=f"e{g}")
                     for g in range(4)]
                for g in range(4):
                    nc.scalar.activation(e[g][:], beta[g][:], Act.Exp)
                eTs = bigp.tile([128, O_LOC, NCH, B], dt.bfloat16,
                                tag="p")
                for o in range(O_LOC):
                    g, h = o % 4, o // 4
                    for ic in range(NCH):
                        eT = psp.tile([128, B], dt.bfloat16, tag="eT",
                                      name=f"eTp{it}_{o}_{ic}")
                        nc.tensor.transpose(
                            eT[:],
                            e[g][64 * h:64 * h + 64,
                                 128 * ic:128 * (ic + 1)],
                            idb[64 * h:64 * h + 64, :])
                        nc.scalar.activation(eTs[:, o, ic, :], eT[:],
                                             Act.Copy)
                zpart = smallp.tile([B, N_IN], dt.float32, tag="zpart")
                for zc in range(3):
                    zps = psp.tile([B, 384], dt.float32, tag="z")
                    for g in range(4):
                        nc.tensor.matmul(
                            zps[:], sel[:],
                            e[g][:, 384 * zc:384 * (zc + 1)],
                            start=(g == 0), stop=(g == 3),
                        )
                    nc.vector.tensor_copy(
                        zpart[:, 384 * zc:384 * (zc + 1)], zps[:])
                nc.sync.dma_start(zin[:], zpart[:])
                nc.gpsimd.collective_compute(
                    "AllReduce", Alu.add,
                    ins=[zin.opt()], outs=[zout.opt()],
                    replica_groups=[list(range(N_CORES))],
                )
                zsb = smallp.tile([B, N_IN], dt.float32, tag="zsb")
                nc.sync.dma_start(zsb[:], zout[:])
                rz = smallp.tile([B, N_IN], dt.float32, tag="rz")
                nc.vector.reciprocal_approx_fast(rz[:], zsb[:])
                rzT = smallp.tile([128, NCH, B], dt.bfloat16, tag="rzT")
                for ic in range(NCH):
                    rzp = psp.tile([128, B], dt.float32, tag="rzp")
                    nc.tensor.transpose(
                        rzp[:], rz[:, 128 * ic:128 * (ic + 1)],
                        idf[:64, :64])
                    nc.vector.tensor_copy(rzT[:, ic, :], rzp[:])

                # ----- cT, y, s-matmuls -----
                psY = psS.tile([128, 2, B], dt.float32, tag="sps")
                n_acc = NCH * D_IN
                for o in range(O_LOC):
                    g, h = o % 4, o // 4
                    cT = smallp.tile([128, NCH, B], dt.bfloat16, tag="cT")
                    nc.vector.tensor_mul(cT[:], eTs[:, o, :, :], rzT[:])
                    ysb = bigp.tile([128, NCH, D_IN, B], dt.bfloat16,
                                    tag="y", bufs=2)
                    cbc = cT.unsqueeze(2).broadcast_to(
                        [128, NCH, D_IN, B])
                    nc.vector.tensor_mul(ysb[:], xt[:], cbc)
                    k = 0
                    for ic in range(NCH):
                        for di in range(D_IN):
                            nc.tensor.matmul(
                                psY[32 * g:32 * g + 16, h, :],
                                wf[:, di, ic, 16 * o:16 * o + 16],
                                ysb[:, ic, di, :],
                                start=(k == 0), stop=(k == n_acc - 1),
                                tile_position=(0, 32 * g),
                            )
                            k += 1
                squash_from_ps(psY, first=False)

            # ---------- final output ----------
            for h in range(2):
                op = psp.tile([B, 128], dt.float32, tag="tiny2")
                nc.tensor.transpose(op[:], sTf[:, h, :], idf[:])
                ofin = smallp.tile([B, 128], dt.float32, tag="ofin")
                fbc = fT2[64 * h:64 * h + 64, :].unsqueeze(2).broadcast_to(
                    [B, 4, 32])
                nc.vector.tensor_mul(
                    ofin.rearrange("b (o r) -> b o r", o=4),
                    op.rearrange("b (o r) -> b o r", o=4),
                    fbc)
                nc.sync.dma_start(d_out[:, h, :], ofin[:])

    nc.compile()
    return nc


def _host_prep(x, W):
    xtc = np.ascontiguousarray(
        x.transpose(1, 2, 0).reshape(NCH, 128, D_IN, B).astype(bf16))
    xb = np.ascontiguousarray(
        x.transpose(0, 2, 1).reshape(B, KD).astype(bf16))
    sel = np.zeros((128, B), np.float32)
    sel[np.arange(128), np.arange(128) % 64] = 1.0
    sel = sel.astype(bf16)
    onesbd = np.zeros((128, 4), np.float32)
    for g in range(4):
        onesbd[32 * g:32 * g + 16, g] = 1.0
    idf = np.eye(128, dtype=np.float32)
    idb = np.concatenate([np.eye(64), np.eye(64)]).astype(np.float32).astype(bf16)

    in_maps = []
    for c in range(N_CORES):
        Wc = W[c * O_LOC:(c + 1) * O_LOC]
        wfc = np.ascontiguousarray(
            Wc.transpose(1, 3, 0, 2)
            .reshape(NCH, 128, D_IN, O_LOC * D_OUT)
            .transpose(1, 2, 0, 3).astype(bf16))
        wdoc = np.zeros((128, 2, KD), np.float32)
        for g in range(4):
            for h in range(2):
                o = 4 * h + g
                wdoc[32 * g:32 * g + 16, h, :] = (
                    Wc[o].transpose(1, 2, 0).reshape(D_OUT, KD))
        in_maps.append({
            "xt": xtc, "wf": wfc,
            "wdo": np.ascontiguousarray(wdoc.astype(bf16)),
            "xb": xb, "sel": sel, "onesbd": onesbd,
            "identf": idf, "identb": idb,
        })
    return in_maps


def kernel(input, W):
    global last_exec_ns
    _install_ntff_hook()
    from concourse.bass_utils import run_bass_kernel_spmd

    x = np.asarray(input, dtype=np.float32)
    W = np.asarray(W, dtype=np.float32)

    if "nc" not in _CACHE:
        _CACHE["nc"] = _build()
    nc = _CACHE["nc"]

    in_maps = _host_prep(x, W)
    trace = bool(int(os.environ.get("CAPS_TRACE", "0")))
    res = run_bass_kernel_spmd(nc, in_maps, core_ids=list(range(N_CORES)),
                               trace=trace)
    last_exec_ns = res.exec_time_ns

    outs = []
    for c in range(N_CORES):
        y = res.results[c]["yout"].reshape(B, 2, 4, 32)[:, :, :, :16]
        outs.append(y.reshape(B, 8, D_OUT))
    return np.concatenate(outs, axis=1).astype(np.float32)

